# revision 42
# baseline (speedup 1.0000x reference)
"""Trainium2 Bass kernel for NeuralCausalModel (per-variable 3-layer MLP).

Math: wx = x @ A.T; per variable i:
    h1 = relu(cat([x, wx[:,i]]) @ W1[i].T + b1[i])
    h2 = relu(h1 @ W2[i].T + b2[i]);  out[:,i] = h2 @ W3[i] + b3[i]
The concat column is folded into W1 host-side (W1eff = W1[:,:, :V] +
W1[:,:,V:]*A), removing the adjacency matmul and the ragged K=257.

Sharding: V=256 split across 8 cores (32 vars/core), x replicated.

All matmul operands are fp16 (same 1 cycle/row PE rate as f32r, half
the weight DMA traffic, and 2x DVE rate on the fp16 SBUF tensors).

Layer 3 (out[:,i] = W3[i]@h2 + b3) is restructured so the W3 weighting
is FREE: the e-axis of W2/b2/W3 is permuted host-side so each 128-row
tile of h2 is sign-pure in W3, |W3| is folded into layer 2's
activation (ACT computes t = relu(|W3|z + |W3|b2) = |W3|*relu(z+b2)
with a per-partition scale), the per-tile signs become a fixed
add/add/sub/sub/sub DVE tensor_tensor tree (the one mixed tile is
emitted twice with complementary zero-masked scales), and a single
ones-vector matmul per batch half does the partition sum on the PE.
A per-variable polarity sigma (applied as a +-1 scale in the final
bias ACT) keeps the program structure identical for every variable
and every core, so one SPMD program serves all sign patterns: all
input-dependence lives in tensor data, never in access patterns.

Matmul loops are ordered so each stationary tile is loaded once and
used for both batch halves (8 weight loads for L1, 16 for L2 per
variable instead of 16/32) to minimize PE weight-load overhead.
"""

import contextlib

import numpy as np

V, D, B = 256, 512, 1024
NCORES = 8
VL = V // NCORES  # 32 variables per core

import os as _os

MM_DTYPE = _os.environ.get("KERNEL_MM_DTYPE", "f16")

_CACHE = {}


def _np_mm_dtype():
    if MM_DTYPE == "bf16":
        import ml_dtypes

        return ml_dtypes.bfloat16
    if MM_DTYPE == "f16":
        return np.float16
    return np.float32


def _build(reps=1):
    key = (MM_DTYPE, reps)
    if key in _CACHE:
        return _CACHE[key]

    import sys

    if "/opt/trn_rl_repo" not in sys.path:
        sys.path.insert(0, "/opt/trn_rl_repo")

    import concourse.mybir as mybir
    import concourse.tile as tile
    from concourse import bacc

    f32 = mybir.dt.float32
    mdt = {
        "f32r": mybir.dt.float32r,
        "bf16": mybir.dt.bfloat16,
        "f16": mybir.dt.float16,
        "f32": mybir.dt.float32,
    }[MM_DTYPE]

    nc = bacc.Bacc("TRN2", target_bir_lowering=False, debug=False)

    xT = nc.declare_dram_parameter("xT", [V, B], mdt, isOutput=False)
    # all 6 weight tiles of a variable packed per-partition so ONE DMA per
    # variable streams them: cols [0:1024]=W1 (2 k-tiles), [1024:3072]=W2
    # (4 d-tiles); the SP sequencer's serial DMA-issue rate was the binder
    wc = nc.declare_dram_parameter("wc", [VL, 128, 3072], mdt, isOutput=False)
    # per-variable +-1 columns: the ones-matmul stationary carries sigma
    ones = nc.declare_dram_parameter("ones", [128, VL], mdt, isOutput=False)
    b1 = nc.declare_dram_parameter("b1t", [128, 128], f32, isOutput=False)
    # 4 bias columns per variable (tiles t0+, t1 mixed, t2-, t3-);
    # |W3| is folded into w2t host-side
    biA = nc.declare_dram_parameter("biA", [128, VL * 4], f32, isOutput=False)
    # per-partition +-1 signs for the mixed tile t1 (sigma-space)
    sg3 = nc.declare_dram_parameter("sg3", [128, VL], f32, isOutput=False)
    b3 = nc.declare_dram_parameter("b3t", [1, VL], f32, isOutput=False)
    out = nc.declare_dram_parameter("out", [VL, B], f32, isOutput=True)

    Relu = mybir.ActivationFunctionType.Relu
    Ident = mybir.ActivationFunctionType.Identity
    add = mybir.AluOpType.add
    sub = mybir.AluOpType.subtract
    amax = mybir.AluOpType.max
    mult = mybir.AluOpType.mult

    with tile.TileContext(nc) as tc:
        with (
            tc.tile_pool(name="const", bufs=1) as const_pool,
            tc.tile_pool(name="wp", bufs=3) as w_pool,
            tc.tile_pool(name="h1p", bufs=8) as h1_pool,
            tc.tile_pool(name="tp", bufs=10) as t_pool,
            tc.tile_pool(name="accp", bufs=3) as acc_pool,
            tc.tile_pool(name="m3p", bufs=4) as m3_pool,
            tc.tile_pool(name="psp", bufs=3, space="PSUM") as ps_pool,
            tc.tile_pool(name="ps3p", bufs=1, space="PSUM") as ps3_pool,
        ):
            # x tiles issue from the ACT-engine DMA queue so the SP engine
            # can issue variable-0's weight DMAs immediately (SP issues
            # serially; the first matmul needs xt AND w1t0)
            xt0 = const_pool.tile([128, B], mdt, tag="xt0")
            xt1 = const_pool.tile([128, B], mdt, tag="xt1")
            # split x loads so the first L1 matmul (needs cols 0:512 of both
            # halves) can start as early as possible
            nc.scalar.dma_start(xt0[:, 0:512], xT[0:128, 0:512])
            nc.scalar.dma_start(xt1[:, 0:512], xT[128:256, 0:512])
            nc.scalar.dma_start(xt0[:, 512:B], xT[0:128, 512:B])
            nc.scalar.dma_start(xt1[:, 512:B], xT[128:256, 512:B])
            b1sb = const_pool.tile([128, 128], f32, tag="b1sb")
            nc.gpsimd.dma_start(b1sb[:], b1[:])
            # less-urgent constants go via the idle Pool engine's DMA queue
            biAsb = const_pool.tile([128, VL * 4], f32, tag="biAsb")
            nc.gpsimd.dma_start(biAsb[:], biA[:])
            sg3sb = const_pool.tile([128, VL], f32, tag="sg3sb")
            nc.gpsimd.dma_start(sg3sb[:], sg3[:])
            onesb = const_pool.tile([128, VL], mdt, tag="onesb")
            nc.gpsimd.dma_start(onesb[:], ones[:])
            b3sb = const_pool.tile([1, VL], f32, tag="b3sb")
            nc.gpsimd.dma_start(b3sb[:], b3[:])

            def emit_l3(v, acc):
                # partition sum via ones-matmul (stationary carries sigma);
                # both halves land in one 2-bank PSUM tile so a single wide
                # ACT applies the b3 bias
                m3sb = m3_pool.tile([1, B], f32, tag="m3sb", name="m3sb")
                ps3 = ps3_pool.tile([1, B], f32, tag="ps3", name="ps3")
                for bb in range(2):
                    bs = slice(bb * 512, (bb + 1) * 512)
                    nc.tensor.matmul(
                        ps3[0:1, bs],
                        onesb[:, v : v + 1],
                        acc[:, bs],
                        start=True,
                        stop=True,
                    )
                nc.scalar.activation(
                    m3sb[0:1, :],
                    ps3[:],
                    Ident,
                    bias=b3sb[0:1, v : v + 1],
                )
                nc.sync.dma_start(out[v : v + 1, :], m3sb[:])

            def emit_l1(v):
                wsb = w_pool.tile([128, 3072], mdt, tag="wsb", name="wsb")
                nc.sync.dma_start(wsb[:], wc[v])

                # L1: stationary w1t[kk][:,ms] loaded once, both batch
                # halves run against it before switching
                h1t = [
                    h1_pool.tile([128, B], mdt, tag="h1t", name=f"h1t_{k}")
                    for k in range(4)
                ]
                for dd in range(4):
                    # one 2-bank PSUM tile per dd: 4 matmuls fill both batch
                    # halves, ONE wide DVE op drains it (halves the PSUM
                    # bank-handoff semaphore traffic and the DVE op count)
                    ps = ps_pool.tile([128, B], f32, tag="ps", name="ps")
                    for kk, xt in ((0, xt0), (1, xt1)):
                        st = slice(kk * 512 + dd * 128, kk * 512 + (dd + 1) * 128)
                        for bb in range(2):
                            bs = slice(bb * 512, (bb + 1) * 512)
                            nc.tensor.matmul(
                                ps[:, bs],
                                wsb[:, st],
                                xt[:, bs],
                                start=(kk == 0),
                                stop=(kk == 1),
                            )
                    nc.vector.tensor_scalar(
                        h1t[dd][:],
                        ps[:],
                        b1sb[:, v * 4 + dd : v * 4 + dd + 1],
                        0.0,
                        op0=add,
                        op1=amax,
                    )
                return h1t, wsb

            def emit_l2(v, h1t, wsb):
                # L2 + fused |W3|-scaled relu (|W3| rides w2t; all on ACT)
                tt = [
                    t_pool.tile([128, B], mdt, tag="tt", name=f"tt_{k}")
                    for k in range(4)
                ]
                for ee in range(4):
                    col = slice(v * 4 + ee, v * 4 + ee + 1)
                    ps2 = ps_pool.tile([128, B], f32, tag="ps", name="ps")
                    for dd in range(4):
                        st = slice(
                            1024 + dd * 512 + ee * 128,
                            1024 + dd * 512 + (ee + 1) * 128,
                        )
                        for bb in range(2):
                            bs = slice(bb * 512, (bb + 1) * 512)
                            nc.tensor.matmul(
                                ps2[:, bs],
                                wsb[:, st],
                                h1t[dd][:, bs],
                                start=(dd == 0),
                                stop=(dd == 3),
                            )
                    nc.scalar.activation(
                        tt[ee][:],
                        ps2[:],
                        Relu,
                        bias=biAsb[:, col],
                    )

                # signed tile tree: the mixed tile 1 applies its
                # per-partition +-1 vector in one early STT op on DVE
                # (t0/t1 are ready first); the two sign-pure subtracts
                # run on the otherwise-idle Pool engine
                acc = acc_pool.tile([128, B], mdt, tag="acc", name="acc")
                nc.vector.scalar_tensor_tensor(
                    acc[:],
                    tt[1][:],
                    sg3sb[:, v : v + 1],
                    tt[0][:],
                    op0=mult,
                    op1=add,
                )
                nc.gpsimd.tensor_tensor(acc[:], acc[:], tt[2][:], sub)
                nc.gpsimd.tensor_tensor(acc[:], acc[:], tt[3][:], sub)
                return acc

            # software pipeline: iteration v runs L1_v, then L3_{v-2}, then
            # L2_{v-1}. DVE gets a full PE-block of time to drain L1's PSUM
            # before L2 needs h1t, and the tree has ~2 blocks before L3.
            rep_ctx = tc.For_i(0, reps, 1) if reps > 1 else contextlib.nullcontext()
            with rep_ctx:
                pend_l2 = None
                pend_l3 = []
                for v in range(VL):
                    h1t, wsb = emit_l1(v)
                    if len(pend_l3) >= 2:
                        emit_l3(*pend_l3.pop(0))
                    if pend_l2 is not None:
                        pv = pend_l2[0]
                        pend_l3.append((pv, emit_l2(*pend_l2)))
                    pend_l2 = (v, h1t, wsb)
                pv = pend_l2[0]
                pend_l3.append((pv, emit_l2(*pend_l2)))
                for item in pend_l3:
                    emit_l3(*item)

    nc.compile()
    _CACHE[key] = nc
    return nc


def _prep_inputs(x, adjacency, W1, b1, W2, b2, W3, b3):
    mmnp = _np_mm_dtype()
    x = np.asarray(x, np.float32)
    A = np.asarray(adjacency, np.float32)
    W1 = np.asarray(W1, np.float32)
    W2 = np.asarray(W2, np.float32)
    W3 = np.asarray(W3, np.float32)
    b1 = np.asarray(b1, np.float32)
    b2 = np.asarray(b2, np.float32)
    b3 = np.asarray(b3, np.float32)

    W1eff = W1[:, :, :V] + W1[:, :, V : V + 1] * A[:, None, :]
    W1effT = np.ascontiguousarray(W1eff.transpose(0, 2, 1)).astype(mmnp)
    xT = np.ascontiguousarray(x.T).astype(mmnp)

    in_maps = []
    for c in range(NCORES):
        s = slice(c * VL, (c + 1) * VL)
        b1t = np.ascontiguousarray(
            b1[s].reshape(VL, 4, 128).transpose(2, 0, 1).reshape(128, VL * 4)
        )
        wcat = np.empty((VL, 128, 3072), mmnp)
        biA = np.empty((128, VL * 4), np.float32)
        sg3 = np.empty((128, VL), np.float32)
        onesc = np.empty((128, VL), np.float32)
        for j, gv in enumerate(range(c * VL, (c + 1) * VL)):
            w3v = W3[gv]
            pos = np.flatnonzero(w3v >= 0)
            neg = np.flatnonzero(w3v < 0)
            if len(pos) < 256:
                sigma, plus, minus = 1.0, pos, neg
            else:
                sigma, plus, minus = -1.0, neg, pos
            L = len(plus)
            assert 128 <= L <= 256, f"degenerate sign split L={L}"
            r = L - 128
            # tile0 pure plus; tile1 mixed (plus rows < r, minus rows >= r);
            # tiles 2/3 pure minus
            perm = np.concatenate(
                [
                    plus[0:128],
                    plus[128:L],
                    minus[0 : 128 - r],
                    minus[128 - r : 384 - r],
                ]
            )
            assert len(perm) == D
            aw3 = np.abs(w3v[perm])
            # |W3| rides the matmul weights: scale W2's (permuted) e-rows
            w2v = ((W2[gv][perm, :] * aw3[:, None]).T).astype(mmnp)  # [d, e]
            w1v = W1effT[gv]  # [V, D] in mm dtype already
            for kk in range(2):
                wcat[j, :, kk * 512 : (kk + 1) * 512] = w1v[
                    kk * 128 : (kk + 1) * 128, :
                ]
            for dd in range(4):
                wcat[j, :, 1024 + dd * 512 : 1024 + (dd + 1) * 512] = w2v[
                    dd * 128 : (dd + 1) * 128, :
                ]
            ab2 = aw3 * b2[gv][perm]
            for ee in range(4):
                biA[:, j * 4 + ee] = ab2[ee * 128 : (ee + 1) * 128]
            sg3[:r, j] = 1.0
            sg3[r:, j] = -1.0
            onesc[:, j] = sigma
        in_maps.append(
            {
                "xT": xT,
                "wc": wcat,
                "biA": biA,
                "sg3": sg3,
                "b1t": b1t,
                "b3t": np.ascontiguousarray(b3[s].reshape(1, VL)),
                "ones": np.ascontiguousarray(onesc).astype(mmnp),
            }
        )
    return in_maps


def kernel(x, adjacency, W1, b1, W2, b2, W3, b3, _trace=False):
    import sys

    if "/opt/trn_rl_repo" not in sys.path:
        sys.path.insert(0, "/opt/trn_rl_repo")
    from concourse.bass_utils import run_bass_kernel_spmd

    nc = _build()
    in_maps = _prep_inputs(x, adjacency, W1, b1, W2, b2, W3, b3)
    res = run_bass_kernel_spmd(
        nc, in_maps, core_ids=list(range(NCORES)), trace=_trace
    )
    kernel.last_results = res
    outT = np.concatenate([res.results[c]["out"] for c in range(NCORES)], axis=0)
    return np.ascontiguousarray(outT.T.astype(np.float32))


kernel.last_results = None
